# revision 23
# baseline (speedup 1.0000x reference)
"""Trainium2 Bass kernel for ChamferLossSplitPID.

Contract: kernel(**inputs) takes the FULL inputs (target/reco [64,512,4] f32,
in_pid/out_pid [64,512] i32) and returns the full output (loss_nonzero,
loss_zero) as float32 scalars, matching reference().

Strategy (8 NeuronCores, data-parallel over batch, 8 batches per core):
  The loss only needs distances between SAME-pid pairs, so instead of the
  full [N, N] distance matrix we compute only the 4 diagonal blocks of the
  pid-grouped matrix, in both directions: rows = points of pid p packed
  into a 128-partition chunk (zero-padded lhsT columns -> pad rows produce
  exactly 0, so no row masks are needed anywhere), cols = other-side points
  of pid p padded to a fixed S=130 stride (pad cols produce dist^2 = 2^27,
  never a min winner). dist^2 is a K=16 split-bf16 matmul (~1e-5 rel).
  Per (batch, dir) that is 4 matmuls of [16,128]x[16,S] -> one PSUM slot
  each; slots pack 3-per-bank so ONE 4D-AP DVE min-reduce drains a whole
  3-bank tile (9 slots). This cuts both PE and DVE work ~4x vs reducing
  the full matrix: the DVE (the bottleneck engine) reads 16*4*S instead of
  16*4*N+pads elements per partition. Rare pid groups with >128 members
  (P[size>128] ~ 0.2%) get their extra rows patched on the host in fp32.
  Tail per half: relu (DVE) -> sqrt (Act) -> ones-matmul partition sum
  (PE) -> Act copy -> DMA. Per-pid norm sums arrive host-premultiplied
  (norm*mask) and need only one DVE row-reduce. The tiny O(B*pid) epilogue
  (counts, divisions, empty-group branches, means) runs on the host, as
  does all layout prep (grouping, hi/lo splits, padding).

The emitted IR is input-value-independent (fixed group stride S, fixed
128-row chunks), so one SPMD program serves all 8 cores. S is bumped
automatically if some pid group exceeds it (recompile, still correct).
"""

import sys

sys.path.insert(0, "/opt/trn_rl_repo")

import numpy as np

from concourse import bacc, bass, mybir, tile
from concourse.bass_utils import run_bass_kernel_spmd

B, N, D = 64, 512, 4
NCORES = 8
BL = B // NCORES          # batches per core
P = 128                   # partitions
NPID = 4                  # nonzero pid classes
BIG = float(2 ** 27)      # pad-column dist^2 (exact in bf16)
KROWS = 16                # split-bf16 contraction rows
NBD = 2 * BL              # (batch, dir) pairs per core
NSLOT = NBD * NPID        # diagonal blocks per core (64)
F32 = mybir.dt.float32
BF16 = mybir.dt.bfloat16

_PROGRAM_CACHE = {}
_SKIP = set()   # debug: subset of {"tailb_mm", "tailb_copy", "taila", "norm"}


def _plan_tiles(S):
    """PSUM tile schedule: (start_slot, nslots) with a small ramp so the
    first DVE reduce starts after only a few matmuls. Slots are 256-element
    (1KB) regions, two per PSUM bank; a full 8-slot tile is 4 banks."""
    plan = []
    t0 = 0
    ramp = [4, 4]             # slots for the first tiles, then 8-slot tiles
    i = 0
    while t0 < NSLOT:
        ns = min(ramp[i] if i < len(ramp) else 8, NSLOT - t0)
        plan.append((t0, ns))
        t0 += ns
        i += 1
    return plan


def _build_program(S: int):
    """Emit the SPMD Bass program for group stride S. Value-independent."""
    COLS = NPID * S
    W = NPID * P + COLS       # lhsT block then rhs block, per direction
    plan = _plan_tiles(S)
    nc = bacc.Bacc(None)

    # lhsT and rhs for one (dir, batch-pair) share one tensor/DMA so each
    # consuming Matmult carries a single sync wait (PE LW allows only one);
    # pairing batches halves the DMA-config count (565ns of queue-sequencer
    # time each), which gates how early the transfers start.
    d_ab = [[nc.dram_tensor(f"ab{d}_{h}", [KROWS, 2 * W], BF16,
                            kind="ExternalInput")
             for h in range(BL // 2)] for d in range(2)]
    # norm*mask premultiplied on host: row g*BL+b, g in (p1..p4 of in_pid,
    # p1..p4 of out_pid, p0 of out_pid)
    d_nrm = nc.dram_tensor("nrm", [9 * BL, N], F32, kind="ExternalInput")
    d_sums = nc.dram_tensor("sums", [1, NSLOT], F32, kind="ExternalOutput")
    d_ns = nc.dram_tensor("ns", [9 * BL, 1], F32, kind="ExternalOutput")

    with tile.TileContext(nc) as tc:
        with (
            tc.tile_pool(name="const", bufs=1) as const,
            tc.tile_pool(name="work", bufs=2) as work,
            tc.tile_pool(name="psum", bufs=2, space=bass.MemorySpace.PSUM) as psum,
        ):
            tAB = [[const.tile([KROWS, 2 * W], BF16, tag=f"ab{d}_{h}",
                               name=f"tAB{d}_{h}") for h in range(BL // 2)]
                   for d in range(2)]
            # interleave pairs across the Sync and Act HWDGE queues in
            # consumption order: dir-0 pairs first, then dir-1, then norms
            # (only needed once the DVE pipeline is already running).
            tNRM = const.tile([9 * BL, N], F32, tag="nrm")
            for dr in range(2):
                for h in range(BL // 2):
                    eng = nc.sync if h % 2 == 0 else nc.scalar
                    eng.dma_start(tAB[dr][h][:], d_ab[dr][h][:])
            nc.scalar.dma_start(tNRM[:], d_nrm[:])
            tONE = const.tile([P, 1], F32, tag="one")
            nc.vector.memset(tONE[:], 1.0)

            tMS = const.tile([P, NSLOT], F32, tag="ms")   # per-block minima
            tSQ = const.tile([P, NSLOT], F32, tag="sq")   # sqrt'd minima
            tSF = const.tile([1, NSLOT], F32, tag="sf")   # partition sums
            tNS = work.tile([9 * BL, 1], F32, tag="nsout")

            def emit_tile(t0, ns):
                # slot s = dir*32 + batch*4 + group (dir-major: the whole
                # first half only needs the dir-0 DMAs). Slots sit at a
                # uniform 256-element stride so ONE 3D-AP DVE reduce drains
                # the whole tile (the baseline-proven access shape).
                pt = psum.tile([P, ns, 256], F32, tag="dist")
                for i in range(ns):
                    s = t0 + i
                    dr, rem = divmod(s, BL * NPID)
                    b, g = divmod(rem, NPID)
                    tj = tAB[dr][b // 2]
                    base = (b % 2) * W
                    nc.tensor.matmul(
                        pt[:, i, 0:S],
                        tj[:, base + g * P : base + (g + 1) * P],
                        tj[:, base + NPID * P + g * S
                           : base + NPID * P + (g + 1) * S],
                        start=True,
                        stop=True,
                    )
                nc.vector.tensor_reduce(
                    tMS[:, t0 : t0 + ns],
                    pt[:, :, 0:S],
                    axis=mybir.AxisListType.X,
                    op=mybir.AluOpType.min,
                )

            def tail_a(lo, hi):
                # pad rows are exactly 0 (zero lhsT cols) and real minima are
                # far from 0, so relu only guards fp rounding; sqrt(0)=0 means
                # pad rows drop out of the partition sums without any mask.
                if "taila" in _SKIP:
                    return
                nc.vector.tensor_scalar_max(tMS[:, lo:hi], tMS[:, lo:hi], 0.0)
                nc.scalar.activation(
                    tSQ[:, lo:hi], tMS[:, lo:hi], mybir.ActivationFunctionType.Sqrt)

            def tail_b(lo, hi):
                if "tailb_mm" in _SKIP:
                    nc.vector.tensor_scalar_max(tSF[:, lo:hi], tSQ[0:1, lo:hi], 0.0)
                    nc.sync.dma_start(d_sums[:, lo:hi], tSF[:, lo:hi])
                    return
                po = psum.tile([1, hi - lo], F32, tag="dist", name=f"po{lo}")
                nc.tensor.matmul(po[:], tONE[:], tSQ[:, lo:hi], start=True, stop=True)
                if "tailb_copy" in _SKIP:
                    nc.vector.tensor_scalar_max(tSF[:, lo:hi], po[:], -1e30)
                else:
                    nc.scalar.copy(tSF[:, lo:hi], po[:])
                nc.sync.dma_start(d_sums[:, lo:hi], tSF[:, lo:hi])

            # piece boundary at the first tile edge covering half the slots
            acc, half = 0, NSLOT
            for t0, ns in plan:
                acc += ns
                if acc >= NSLOT // 2:
                    half = acc
                    break

            emitted = 0
            pending = []  # deferred tail_b stages: (emit_after_slots, lo, hi)
            for idx, (t0, ns) in enumerate(plan):
                emit_tile(t0, ns)
                emitted += ns
                if emitted == half and emitted < NSLOT:
                    tail_a(0, half)
                    # defer the PE/Act stage ~2 tiles so the PE never stalls
                    pending.append((min(NSLOT, emitted + 18), 0, half))
                if idx == 5 and "norm" not in _SKIP:
                    # norm-sum reduce early, during the DVE pipeline ramp
                    nc.vector.tensor_reduce(
                        tNS[:], tNRM[:], axis=mybir.AxisListType.X,
                        op=mybir.AluOpType.add)
                    nc.sync.dma_start(d_ns[:], tNS[:])
                while pending and emitted >= pending[0][0]:
                    _, lo, hi = pending.pop(0)
                    tail_b(lo, hi)
            tail_a(half, NSLOT)
            for _, lo, hi in pending:
                tail_b(lo, hi)
            tail_b(half, NSLOT)

    nc.compile()
    return nc


def _get_program(S: int):
    if S not in _PROGRAM_CACHE:
        _PROGRAM_CACHE[S] = _build_program(S)
    return _PROGRAM_CACHE[S]


def _prep_inputs(target, reco, in_pid, out_pid, S):
    """Build per-core input maps. All heavy compute stays on device; this is
    O(B*N) metadata/layout prep (grouping, norms, hi/lo splits, padding)."""
    COLS = NPID * S
    W = NPID * P + COLS
    t = np.ascontiguousarray(np.asarray(target, dtype=np.float32))
    r = np.ascontiguousarray(np.asarray(reco, dtype=np.float32))
    ip = np.asarray(in_pid)
    op = np.asarray(out_pid)

    import ml_dtypes

    def split16(x):
        hi = x.astype(ml_dtypes.bfloat16).astype(np.float32)
        lo = (x - hi).astype(ml_dtypes.bfloat16).astype(np.float32)
        return hi, lo

    nt2 = (t * t).sum(-1)                      # [B,N]
    nr2 = (r * r).sum(-1)
    ones = np.ones((B, 1, N), np.float32)
    # split-bf16: a.b ~= ahi.bhi + ahi.blo + alo.bhi (lo.lo dropped, ~2^-16 rel)
    # lhsT rows: [(-2x)hi x4, (-2x)hi x4, (-2x)lo x4, |x|2hi, |x|2lo, 1, 1]
    # rhs rows:  [ yhi x4,     ylo x4,     yhi x4,    1,      1, |y|2hi, |y|2lo]
    def build_lhs(x, x2):
        m2hi, m2lo = split16(-2.0 * x.transpose(0, 2, 1))   # [B,4,N]
        x2hi, x2lo = split16(x2[:, None, :])                # [B,1,N]
        return np.concatenate(
            [m2hi, m2hi, m2lo, x2hi, x2lo, ones, ones], axis=1)  # [B,16,N]

    Lt = build_lhs(t, nt2)
    Lr = build_lhs(r, nr2)
    thi, tlo = split16(t)
    rhi, rlo = split16(r)
    t2hi, t2lo = split16(nt2)
    r2hi, r2lo = split16(nr2)

    AB = np.zeros((2, B, KROWS, W), np.float32)
    sides = [(Lt, ip, rhi, rlo, r2hi, r2lo, op),   # dir0: rows targets, cols recos
             (Lr, op, thi, tlo, t2hi, t2lo, ip)]   # dir1: rows recos, cols targets
    for dirn, (xL, xpid, yhi, ylo, y2hi, y2lo, ypid) in enumerate(sides):
        for b in range(B):
            for g in range(NPID):
                p = g + 1
                ridx = np.nonzero(xpid[b] == p)[0][:P]
                AB[dirn, b, :, g * P : g * P + len(ridx)] = xL[b][:, ridx]
                cidx = np.nonzero(ypid[b] == p)[0]
                c0 = NPID * P + g * S
                k = len(cidx)
                AB[dirn, b, 0:4, c0 : c0 + k] = yhi[b, cidx].T
                AB[dirn, b, 4:8, c0 : c0 + k] = ylo[b, cidx].T
                AB[dirn, b, 8:12, c0 : c0 + k] = yhi[b, cidx].T
                AB[dirn, b, 12:14, c0 : c0 + k] = 1.0
                AB[dirn, b, 14, c0 : c0 + k] = y2hi[b, cidx]
                AB[dirn, b, 15, c0 : c0 + k] = y2lo[b, cidx]
                AB[dirn, b, 14, c0 + k : c0 + S] = BIG

    normt = np.sqrt(nt2).astype(np.float32)
    normr = np.sqrt(nr2).astype(np.float32)
    # 72-row layout, premultiplied: row g*BL+b; g 0-3: normt*(in_pid==g+1);
    # g 4-7: normr*(out_pid==g-3); g 8: normr*(out_pid==0)
    grp = [normt * (ip == p) for p in (1, 2, 3, 4)]
    grp += [normr * (op == p) for p in (1, 2, 3, 4, 0)]

    in_maps = []
    for ci in range(NCORES):
        s = slice(ci * BL, (ci + 1) * BL)
        m = {"nrm": np.ascontiguousarray(
            np.concatenate([g[s] for g in grp], axis=0))}
        for dirn in range(2):
            for h in range(BL // 2):
                pair = np.concatenate(
                    [AB[dirn, ci * BL + 2 * h], AB[dirn, ci * BL + 2 * h + 1]],
                    axis=1)
                m[f"ab{dirn}_{h}"] = np.ascontiguousarray(
                    pair.astype(ml_dtypes.bfloat16))
        in_maps.append(m)
    return in_maps


def _overflow_corrections(t, r, ip, op):
    """fp32 host patch for pid groups with >128 members: device rows are
    capped at 128 partitions, the few extra rows' min-distances are added
    here. O(overflow_rows * S) — expected ~zero rows per input."""
    corr_xy = np.zeros((B, 5))
    corr_yx = np.zeros((B, 5))
    for b in range(B):
        for p in range(1, 5):
            ridx = np.nonzero(ip[b] == p)[0]
            cidx = np.nonzero(op[b] == p)[0]
            if len(ridx) > P and len(cidx) > 0:
                for i in ridx[P:]:
                    d2 = ((t[b, i][None, :] - r[b, cidx]) ** 2).sum(-1)
                    corr_xy[b, p] += np.sqrt(d2.min())
            if len(cidx) > P and len(ridx) > 0:
                for j in cidx[P:]:
                    d2 = ((r[b, j][None, :] - t[b, ridx]) ** 2).sum(-1)
                    corr_yx[b, p] += np.sqrt(d2.min())
    return corr_xy, corr_yx


def _epilogue(sums_all, ns_all, t, r, ip, op):
    """Tiny O(B*pid) final combination, mirrors reference()'s branch logic."""
    sum_xy = np.zeros((B, 5))
    sum_yx = np.zeros((B, 5))
    only_x = np.zeros((B, 5))
    only_y = np.zeros((B, 5))
    zerosum = np.zeros(B)
    for ci in range(NCORES):
        srow = sums_all[ci].reshape(2, BL, NPID)   # slot = dir*32 + b*4 + g
        ns72 = ns_all[ci].reshape(9, BL)
        for lb in range(BL):
            b = ci * BL + lb
            sum_xy[b, 1:5] = srow[0, lb]
            sum_yx[b, 1:5] = srow[1, lb]
            only_x[b, 1:5] = ns72[0:4, lb]
            only_y[b, 1:5] = ns72[4:8, lb]
            zerosum[b] = ns72[8, lb]

    cxy, cyx = _overflow_corrections(t, r, ip, op)
    sum_xy += cxy
    sum_yx += cyx

    cx = np.stack([(ip == p).sum(1) for p in range(5)], 1)  # [B,5]
    cy = np.stack([(op == p).sum(1) for p in range(5)], 1)

    loss_nonzero = np.float32(0.0)
    for p in range(1, 5):
        both = 0.5 * (sum_xy[:, p] / np.maximum(1, cy[:, p])
                      + sum_yx[:, p] / np.maximum(1, cx[:, p]))
        ox = only_x[:, p] / np.maximum(1, cx[:, p])
        oy = only_y[:, p] / np.maximum(1, cy[:, p])
        per_b = np.where(cy[:, p] == 0, ox, np.where(cx[:, p] == 0, oy, both))
        loss_nonzero = loss_nonzero + np.float32(per_b.mean())
    loss_zero = np.float32((zerosum / np.maximum(1, cy[:, 0])).mean())
    return np.float32(loss_nonzero), np.float32(loss_zero)


def kernel(target, reco, in_pid, out_pid):
    t = np.ascontiguousarray(np.asarray(target, dtype=np.float32))
    r = np.ascontiguousarray(np.asarray(reco, dtype=np.float32))
    ip = np.asarray(in_pid)
    op = np.asarray(out_pid)
    # fixed group stride; bump (recompile) only if a pid group overflows it
    max_grp = 0
    for pid in (ip, op):
        for p in range(1, 5):
            max_grp = max(max_grp, int((pid == p).sum(1).max()))
    S = 130
    while S < max_grp:
        S += 8
    S = min(S, 512)  # a pid group can never exceed N=512

    nc = _get_program(S)
    in_maps = _prep_inputs(t, r, ip, op, S)
    res = run_bass_kernel_spmd(nc, in_maps, list(range(NCORES)))
    sums_all = [res.results[ci]["sums"] for ci in range(NCORES)]
    ns_all = [res.results[ci]["ns"] for ci in range(NCORES)]
    return _epilogue(sums_all, ns_all, t, r, ip, op)


# revision 24
# speedup vs baseline: 1.1223x; 1.1223x over previous
"""Trainium2 Bass kernel for ChamferLossSplitPID.

Contract: kernel(**inputs) takes the FULL inputs (target/reco [64,512,4] f32,
in_pid/out_pid [64,512] i32) and returns the full output (loss_nonzero,
loss_zero) as float32 scalars, matching reference().

Strategy (8 NeuronCores, data-parallel over batch, 8 batches per core):
  The loss only needs distances between SAME-pid pairs, so instead of the
  full [N, N] distance matrix we compute only the 4 diagonal blocks of the
  pid-grouped matrix, in both directions: rows = points of pid p packed
  into a 128-partition chunk (zero-padded lhsT columns -> pad rows produce
  exactly 0, so no row masks are needed anywhere), cols = other-side points
  of pid p padded to a fixed S=130 stride (pad cols produce dist^2 = 2^27,
  never a min winner). dist^2 is a K=16 split-bf16 matmul (~1e-5 rel).
  Per (batch, dir) that is 4 matmuls of [16,128]x[16,S] -> one PSUM slot
  each; slots pack 3-per-bank so ONE 4D-AP DVE min-reduce drains a whole
  3-bank tile (9 slots). This cuts both PE and DVE work ~4x vs reducing
  the full matrix: the DVE (the bottleneck engine) reads 16*4*S instead of
  16*4*N+pads elements per partition. Rare pid groups with >128 members
  (P[size>128] ~ 0.2%) get their extra rows patched on the host in fp32.
  Tail per half: relu (DVE) -> sqrt (Act) -> ones-matmul partition sum
  (PE) -> Act copy -> DMA. Per-pid norm sums arrive host-premultiplied
  (norm*mask) and need only one DVE row-reduce. The tiny O(B*pid) epilogue
  (counts, divisions, empty-group branches, means) runs on the host, as
  does all layout prep (grouping, hi/lo splits, padding).

The emitted IR is input-value-independent (fixed group stride S, fixed
128-row chunks), so one SPMD program serves all 8 cores. S is bumped
automatically if some pid group exceeds it (recompile, still correct).
"""

import sys

sys.path.insert(0, "/opt/trn_rl_repo")

import numpy as np

from concourse import bacc, bass, mybir, tile
from concourse.bass_utils import run_bass_kernel_spmd

B, N, D = 64, 512, 4
NCORES = 8
BL = B // NCORES          # batches per core
P = 128                   # partitions
NPID = 4                  # nonzero pid classes
BIG = float(2 ** 27)      # pad-column dist^2 (exact in bf16)
KROWS = 16                # split-bf16 contraction rows
NBD = 2 * BL              # (batch, dir) pairs per core
NSLOT = NBD * NPID        # diagonal blocks per core (64)
F32 = mybir.dt.float32
BF16 = mybir.dt.bfloat16

_PROGRAM_CACHE = {}
_SKIP = set()   # debug: subset of {"tailb_mm", "tailb_copy", "taila", "norm"}


def _plan_tiles(S):
    """PSUM tile schedule: (start_slot, nslots) with a small ramp so the
    first DVE reduce starts after only a few matmuls. Slots are 256-element
    (1KB) regions, two per PSUM bank; a full 8-slot tile is 4 banks."""
    plan = []
    t0 = 0
    ramp = [4, 4]             # slots for the first tiles, then 8-slot tiles
    i = 0
    while t0 < NSLOT:
        ns = min(ramp[i] if i < len(ramp) else 8, NSLOT - t0)
        plan.append((t0, ns))
        t0 += ns
        i += 1
    return plan


def _build_program(S: int):
    """Emit the SPMD Bass program for group stride S. Value-independent."""
    COLS = NPID * S
    W = NPID * P + COLS       # lhsT block then rhs block, per direction
    plan = _plan_tiles(S)
    nc = bacc.Bacc(None)

    # lhsT and rhs for one (dir, batch) share one tensor/DMA so each
    # consuming Matmult carries a single sync wait (PE LW allows only one).
    d_ab = [[nc.dram_tensor(f"ab{d}_{b}", [KROWS, W], BF16, kind="ExternalInput")
             for b in range(BL)] for d in range(2)]
    # norm*mask premultiplied on host: row g*BL+b, g in (p1..p4 of in_pid,
    # p1..p4 of out_pid, p0 of out_pid)
    d_nrm = nc.dram_tensor("nrm", [9 * BL, N], F32, kind="ExternalInput")
    d_sums = nc.dram_tensor("sums", [1, NSLOT], F32, kind="ExternalOutput")
    d_ns = nc.dram_tensor("ns", [9 * BL, 1], F32, kind="ExternalOutput")

    with tile.TileContext(nc) as tc:
        with (
            tc.tile_pool(name="const", bufs=1) as const,
            tc.tile_pool(name="work", bufs=2) as work,
            tc.tile_pool(name="psum", bufs=2, space=bass.MemorySpace.PSUM) as psum,
        ):
            tAB = [[const.tile([KROWS, W], BF16, tag=f"ab{d}_{b}",
                               name=f"tAB{d}_{b}") for b in range(BL)]
                   for d in range(2)]
            # interleave batches across the Sync and Act HWDGE queues in
            # consumption order: dir-0 batches first, then dir-1, then norms
            # (only needed once the DVE pipeline is already running).
            tNRM = const.tile([9 * BL, N], F32, tag="nrm")
            for dr in range(2):
                for b in range(BL):
                    eng = nc.sync if b % 2 == 0 else nc.scalar
                    eng.dma_start(tAB[dr][b][:], d_ab[dr][b][:])
            nc.scalar.dma_start(tNRM[:], d_nrm[:])
            tONE = const.tile([P, 1], F32, tag="one")
            nc.vector.memset(tONE[:], 1.0)

            tMS = const.tile([P, NSLOT], F32, tag="ms")   # per-block minima
            tSQ = const.tile([P, NSLOT], F32, tag="sq")   # sqrt'd minima
            tSF = const.tile([1, NSLOT], F32, tag="sf")   # partition sums
            tNS = work.tile([9 * BL, 1], F32, tag="nsout")

            def emit_tile(t0, ns):
                # slot s = dir*32 + batch*4 + group (dir-major: the whole
                # first half only needs the dir-0 DMAs). Slots sit at a
                # uniform 256-element stride so ONE 3D-AP DVE reduce drains
                # the whole tile (the baseline-proven access shape).
                pt = psum.tile([P, ns, 256], F32, tag="dist")
                for i in range(ns):
                    s = t0 + i
                    dr, rem = divmod(s, BL * NPID)
                    b, g = divmod(rem, NPID)
                    nc.tensor.matmul(
                        pt[:, i, 0:S],
                        tAB[dr][b][:, g * P : (g + 1) * P],
                        tAB[dr][b][:, NPID * P + g * S : NPID * P + (g + 1) * S],
                        start=True,
                        stop=True,
                    )
                nc.vector.tensor_reduce(
                    tMS[:, t0 : t0 + ns],
                    pt[:, :, 0:S],
                    axis=mybir.AxisListType.X,
                    op=mybir.AluOpType.min,
                )

            def tail_a(lo, hi):
                # pad rows are exactly 0 (zero lhsT cols) and real minima are
                # far from 0, so relu only guards fp rounding; sqrt(0)=0 means
                # pad rows drop out of the partition sums without any mask.
                if "taila" in _SKIP:
                    return
                nc.vector.tensor_scalar_max(tMS[:, lo:hi], tMS[:, lo:hi], 0.0)
                nc.scalar.activation(
                    tSQ[:, lo:hi], tMS[:, lo:hi], mybir.ActivationFunctionType.Sqrt)

            def tail_b(lo, hi):
                if "tailb_mm" in _SKIP:
                    nc.vector.tensor_scalar_max(tSF[:, lo:hi], tSQ[0:1, lo:hi], 0.0)
                    nc.sync.dma_start(d_sums[:, lo:hi], tSF[:, lo:hi])
                    return
                po = psum.tile([1, hi - lo], F32, tag="dist", name=f"po{lo}")
                nc.tensor.matmul(po[:], tONE[:], tSQ[:, lo:hi], start=True, stop=True)
                if "tailb_copy" in _SKIP:
                    nc.vector.tensor_scalar_max(tSF[:, lo:hi], po[:], -1e30)
                else:
                    nc.scalar.copy(tSF[:, lo:hi], po[:])
                nc.sync.dma_start(d_sums[:, lo:hi], tSF[:, lo:hi])

            # piece boundary at the first tile edge covering half the slots
            acc, half = 0, NSLOT
            for t0, ns in plan:
                acc += ns
                if acc >= NSLOT // 2:
                    half = acc
                    break

            emitted = 0
            pending = []  # deferred tail_b stages: (emit_after_slots, lo, hi)
            for idx, (t0, ns) in enumerate(plan):
                emit_tile(t0, ns)
                emitted += ns
                if emitted == half and emitted < NSLOT:
                    tail_a(0, half)
                    # defer the PE/Act stage ~2 tiles so the PE never stalls
                    pending.append((min(NSLOT, emitted + 18), 0, half))
                if idx == 5 and "norm" not in _SKIP:
                    # norm-sum reduce early, during the DVE pipeline ramp
                    nc.vector.tensor_reduce(
                        tNS[:], tNRM[:], axis=mybir.AxisListType.X,
                        op=mybir.AluOpType.add)
                    nc.sync.dma_start(d_ns[:], tNS[:])
                while pending and emitted >= pending[0][0]:
                    _, lo, hi = pending.pop(0)
                    tail_b(lo, hi)
            tail_a(half, NSLOT)
            for _, lo, hi in pending:
                tail_b(lo, hi)
            tail_b(half, NSLOT)

    nc.compile()
    return nc


def _get_program(S: int):
    if S not in _PROGRAM_CACHE:
        _PROGRAM_CACHE[S] = _build_program(S)
    return _PROGRAM_CACHE[S]


def _prep_inputs(target, reco, in_pid, out_pid, S):
    """Build per-core input maps. All heavy compute stays on device; this is
    O(B*N) metadata/layout prep (grouping, norms, hi/lo splits, padding)."""
    COLS = NPID * S
    W = NPID * P + COLS
    t = np.ascontiguousarray(np.asarray(target, dtype=np.float32))
    r = np.ascontiguousarray(np.asarray(reco, dtype=np.float32))
    ip = np.asarray(in_pid)
    op = np.asarray(out_pid)

    import ml_dtypes

    def split16(x):
        hi = x.astype(ml_dtypes.bfloat16).astype(np.float32)
        lo = (x - hi).astype(ml_dtypes.bfloat16).astype(np.float32)
        return hi, lo

    nt2 = (t * t).sum(-1)                      # [B,N]
    nr2 = (r * r).sum(-1)
    ones = np.ones((B, 1, N), np.float32)
    # split-bf16: a.b ~= ahi.bhi + ahi.blo + alo.bhi (lo.lo dropped, ~2^-16 rel)
    # lhsT rows: [(-2x)hi x4, (-2x)hi x4, (-2x)lo x4, |x|2hi, |x|2lo, 1, 1]
    # rhs rows:  [ yhi x4,     ylo x4,     yhi x4,    1,      1, |y|2hi, |y|2lo]
    def build_lhs(x, x2):
        m2hi, m2lo = split16(-2.0 * x.transpose(0, 2, 1))   # [B,4,N]
        x2hi, x2lo = split16(x2[:, None, :])                # [B,1,N]
        return np.concatenate(
            [m2hi, m2hi, m2lo, x2hi, x2lo, ones, ones], axis=1)  # [B,16,N]

    Lt = build_lhs(t, nt2)
    Lr = build_lhs(r, nr2)
    thi, tlo = split16(t)
    rhi, rlo = split16(r)
    t2hi, t2lo = split16(nt2)
    r2hi, r2lo = split16(nr2)

    AB = np.zeros((2, B, KROWS, W), np.float32)
    sides = [(Lt, ip, rhi, rlo, r2hi, r2lo, op),   # dir0: rows targets, cols recos
             (Lr, op, thi, tlo, t2hi, t2lo, ip)]   # dir1: rows recos, cols targets
    for dirn, (xL, xpid, yhi, ylo, y2hi, y2lo, ypid) in enumerate(sides):
        for b in range(B):
            for g in range(NPID):
                p = g + 1
                ridx = np.nonzero(xpid[b] == p)[0][:P]
                AB[dirn, b, :, g * P : g * P + len(ridx)] = xL[b][:, ridx]
                cidx = np.nonzero(ypid[b] == p)[0]
                c0 = NPID * P + g * S
                k = len(cidx)
                AB[dirn, b, 0:4, c0 : c0 + k] = yhi[b, cidx].T
                AB[dirn, b, 4:8, c0 : c0 + k] = ylo[b, cidx].T
                AB[dirn, b, 8:12, c0 : c0 + k] = yhi[b, cidx].T
                AB[dirn, b, 12:14, c0 : c0 + k] = 1.0
                AB[dirn, b, 14, c0 : c0 + k] = y2hi[b, cidx]
                AB[dirn, b, 15, c0 : c0 + k] = y2lo[b, cidx]
                AB[dirn, b, 14, c0 + k : c0 + S] = BIG

    normt = np.sqrt(nt2).astype(np.float32)
    normr = np.sqrt(nr2).astype(np.float32)
    # 72-row layout, premultiplied: row g*BL+b; g 0-3: normt*(in_pid==g+1);
    # g 4-7: normr*(out_pid==g-3); g 8: normr*(out_pid==0)
    grp = [normt * (ip == p) for p in (1, 2, 3, 4)]
    grp += [normr * (op == p) for p in (1, 2, 3, 4, 0)]

    in_maps = []
    for ci in range(NCORES):
        s = slice(ci * BL, (ci + 1) * BL)
        m = {"nrm": np.ascontiguousarray(
            np.concatenate([g[s] for g in grp], axis=0))}
        for dirn in range(2):
            for b in range(BL):
                m[f"ab{dirn}_{b}"] = np.ascontiguousarray(
                    AB[dirn, ci * BL + b].astype(ml_dtypes.bfloat16))
        in_maps.append(m)
    return in_maps


def _overflow_corrections(t, r, ip, op):
    """fp32 host patch for pid groups with >128 members: device rows are
    capped at 128 partitions, the few extra rows' min-distances are added
    here. O(overflow_rows * S) — expected ~zero rows per input."""
    corr_xy = np.zeros((B, 5))
    corr_yx = np.zeros((B, 5))
    for b in range(B):
        for p in range(1, 5):
            ridx = np.nonzero(ip[b] == p)[0]
            cidx = np.nonzero(op[b] == p)[0]
            if len(ridx) > P and len(cidx) > 0:
                for i in ridx[P:]:
                    d2 = ((t[b, i][None, :] - r[b, cidx]) ** 2).sum(-1)
                    corr_xy[b, p] += np.sqrt(d2.min())
            if len(cidx) > P and len(ridx) > 0:
                for j in cidx[P:]:
                    d2 = ((r[b, j][None, :] - t[b, ridx]) ** 2).sum(-1)
                    corr_yx[b, p] += np.sqrt(d2.min())
    return corr_xy, corr_yx


def _epilogue(sums_all, ns_all, t, r, ip, op):
    """Tiny O(B*pid) final combination, mirrors reference()'s branch logic."""
    sum_xy = np.zeros((B, 5))
    sum_yx = np.zeros((B, 5))
    only_x = np.zeros((B, 5))
    only_y = np.zeros((B, 5))
    zerosum = np.zeros(B)
    for ci in range(NCORES):
        srow = sums_all[ci].reshape(2, BL, NPID)   # slot = dir*32 + b*4 + g
        ns72 = ns_all[ci].reshape(9, BL)
        for lb in range(BL):
            b = ci * BL + lb
            sum_xy[b, 1:5] = srow[0, lb]
            sum_yx[b, 1:5] = srow[1, lb]
            only_x[b, 1:5] = ns72[0:4, lb]
            only_y[b, 1:5] = ns72[4:8, lb]
            zerosum[b] = ns72[8, lb]

    cxy, cyx = _overflow_corrections(t, r, ip, op)
    sum_xy += cxy
    sum_yx += cyx

    cx = np.stack([(ip == p).sum(1) for p in range(5)], 1)  # [B,5]
    cy = np.stack([(op == p).sum(1) for p in range(5)], 1)

    loss_nonzero = np.float32(0.0)
    for p in range(1, 5):
        both = 0.5 * (sum_xy[:, p] / np.maximum(1, cy[:, p])
                      + sum_yx[:, p] / np.maximum(1, cx[:, p]))
        ox = only_x[:, p] / np.maximum(1, cx[:, p])
        oy = only_y[:, p] / np.maximum(1, cy[:, p])
        per_b = np.where(cy[:, p] == 0, ox, np.where(cx[:, p] == 0, oy, both))
        loss_nonzero = loss_nonzero + np.float32(per_b.mean())
    loss_zero = np.float32((zerosum / np.maximum(1, cy[:, 0])).mean())
    return np.float32(loss_nonzero), np.float32(loss_zero)


def kernel(target, reco, in_pid, out_pid):
    t = np.ascontiguousarray(np.asarray(target, dtype=np.float32))
    r = np.ascontiguousarray(np.asarray(reco, dtype=np.float32))
    ip = np.asarray(in_pid)
    op = np.asarray(out_pid)
    # fixed group stride; bump (recompile) only if a pid group overflows it
    max_grp = 0
    for pid in (ip, op):
        for p in range(1, 5):
            max_grp = max(max_grp, int((pid == p).sum(1).max()))
    S = 130
    while S < max_grp:
        S += 8
    S = min(S, 512)  # a pid group can never exceed N=512

    nc = _get_program(S)
    in_maps = _prep_inputs(t, r, ip, op, S)
    res = run_bass_kernel_spmd(nc, in_maps, list(range(NCORES)))
    sums_all = [res.results[ci]["sums"] for ci in range(NCORES)]
    ns_all = [res.results[ci]["ns"] for ci in range(NCORES)]
    return _epilogue(sums_all, ns_all, t, r, ip, op)
